# revision 1
# baseline (speedup 1.0000x reference)
"""Trainium2 Bass kernel for 3-layer per-task LoRA MLP.

Full-input contract: kernel(**inputs) takes the unsharded tensors and returns
the full [8, 1024, 1024] output. Internally the task axis (t=8) is sharded
across 8 NeuronCores (one task per core); base weights are replicated.

Per-core layout strategy:
  - activations live transposed in SBUF: h^T [feat(part), batch(free)]
  - base weights k0/k1 stream in natural [K, M] layout as the matmul
    stationary operand; moving operand is the transposed activation
  - LoRA: z^T = (scaling*d)^T-contraction matmul, then the rank-8 delta is
    one extra accumulating matmul into the same PSUM group as the base
  - final layer uses h2^T as the *stationary* operand and k2 as the moving
    operand, producing natural-layout [batch, feat] output directly
  - fp32 bits are bitcast to float32r at matmul sites => 1 cycle/row (4x
    over plain fp32) for N>=256
"""

import sys

if "/opt/trn_rl_repo" not in sys.path:
    sys.path.insert(0, "/opt/trn_rl_repo")

import numpy as np

T, B, D = 8, 1024, 1024
H1, H2, H3 = 2048, 2048, 1024
R = 8
SCALING = 2.0  # alpha/rank = 16/8
P = 128
NT = 512  # PSUM free-dim tile (fp32 one-bank limit)

_CACHE = {}


def _build(mm_mode="f32r"):
    import concourse.bass as bass
    import concourse.mybir as mybir
    from concourse import bacc
    from concourse.tile import TileContext
    from concourse.bass import ts
    from concourse.masks import make_identity

    f32 = mybir.dt.float32
    f32r = mybir.dt.float32r
    AF = mybir.ActivationFunctionType

    fmm = f32r if mm_mode == "f32r" else f32

    def mc(ap):
        return ap

    nc = bacc.Bacc(None, target_bir_lowering=False, name="lora_mlp")

    x = nc.dram_tensor("x", (B, D), f32, kind="ExternalInput")
    k0 = nc.dram_tensor("k0", (D, H1), fmm, kind="ExternalInput")
    b0 = nc.dram_tensor("b0", (H1,), f32, kind="ExternalInput")
    d0 = nc.dram_tensor("d0", (D, R), fmm, kind="ExternalInput")
    u0 = nc.dram_tensor("u0", (R, H1), fmm, kind="ExternalInput")
    k1 = nc.dram_tensor("k1", (H1, H2), fmm, kind="ExternalInput")
    b1 = nc.dram_tensor("b1", (H2,), f32, kind="ExternalInput")
    d1 = nc.dram_tensor("d1", (H1, R), fmm, kind="ExternalInput")
    u1 = nc.dram_tensor("u1", (R, H2), fmm, kind="ExternalInput")
    k2 = nc.dram_tensor("k2", (H2, H3), fmm, kind="ExternalInput")
    b2 = nc.dram_tensor("b2", (H3,), fmm, kind="ExternalInput")
    d2 = nc.dram_tensor("d2", (H2, R), fmm, kind="ExternalInput")
    u2 = nc.dram_tensor("u2", (R, H3), fmm, kind="ExternalInput")
    out = nc.dram_tensor("out", (B, H3), f32, kind="ExternalOutput")

    KT0 = D // P      # 8  k-tiles, layer 0
    KT1 = H1 // P     # 16 k-tiles, layer 1
    KT2 = H2 // P     # 16 k-tiles, layer 2
    MT0 = H1 // P     # 16 m-tiles, layer 0
    MT1 = H2 // P     # 16 m-tiles, layer 1
    BT = B // P       # 8  batch 128-tiles
    NB = B // NT      # 2  batch 512-halves (free dim, layers 0/1)
    N2 = H3 // NT     # 2  feature 512-halves (free dim, layer 2)
    KG2 = 4           # layer-2 k-group size (k2 streamed in groups)

    with TileContext(nc) as tc:
        with (
            tc.tile_pool(name="main", bufs=1) as pool,
            tc.tile_pool(name="psum", bufs=1, space="PSUM") as pp,
        ):
            ident = pool.tile([P, P], f32, tag="ident", bufs=1)
            make_identity(nc, ident)
            ones_f = pool.tile([1, P], f32, tag="ones_f", bufs=1)
            nc.vector.memset(ones_f, 1.0)
            ones = pool.tile([1, P], fmm, tag="ones", bufs=1)
            nc.vector.tensor_copy(ones, ones_f)

            # small constants: lora d (pre-scaled on host), u, biases
            d0_sb = pool.tile([P, KT0 * R], fmm, tag="d0", bufs=1)
            nc.sync.dma_start(
                out=d0_sb.rearrange("p (k r) -> p k r", r=R),
                in_=d0[:, :].rearrange("(k p) r -> p k r", p=P),
            )
            d1_sb = pool.tile([P, KT1 * R], fmm, tag="d1", bufs=1)
            nc.sync.dma_start(
                out=d1_sb.rearrange("p (k r) -> p k r", r=R),
                in_=d1[:, :].rearrange("(k p) r -> p k r", p=P),
            )
            d2_sb = pool.tile([P, KT2 * R], fmm, tag="d2", bufs=1)
            nc.sync.dma_start(
                out=d2_sb.rearrange("p (k r) -> p k r", r=R),
                in_=d2[:, :].rearrange("(k p) r -> p k r", p=P),
            )
            u0_sb = pool.tile([R, H1], fmm, tag="u", bufs=1)
            nc.sync.dma_start(out=u0_sb, in_=u0[:, :])

            b0_sb = pool.tile([P, MT0], f32, tag="b0", bufs=1)
            for m in range(MT0):
                nc.sync.dma_start(
                    out=b0_sb[:, ts(m, 1)], in_=b0[ts(m, P)].unsqueeze(1)
                )
            b1_sb = pool.tile([P, MT1], f32, tag="b1", bufs=1)
            for m in range(MT1):
                nc.sync.dma_start(
                    out=b1_sb[:, ts(m, 1)], in_=b1[ts(m, P)].unsqueeze(1)
                )
            b2_sb = pool.tile([1, H3], fmm, tag="b2", bufs=1)
            nc.sync.dma_start(out=b2_sb, in_=b2[:].unsqueeze(0))

            # ---- load x and transpose to xT [D(part), B(free)] ----
            xT = []
            for di in range(KT0):
                xT.append(pool.tile([P, B], fmm, tag="E", bufs=8, name=f"xT{di}"))
            for bi in range(BT):
                xn = pool.tile([P, D], f32, tag="xn", bufs=3)
                nc.sync.dma_start(out=xn, in_=x[ts(bi, P), :])
                for di in range(KT0):
                    pt = pp.tile([P, P], f32, tag="pt", bufs=2)
                    nc.tensor.transpose(pt, xn[:, ts(di, P)], ident)
                    nc.vector.tensor_copy(xT[di][:, ts(bi, P)], pt)

            def lora_zT(d_sb, kt, src_tiles, tag):
                """z^T [R, B] = (scaling*d)^T @ h  via PSUM accumulation."""
                z_sb = pool.tile([R, B], fmm, tag=tag, bufs=1)
                for n in range(NB):
                    pz = pp.tile([R, NT], f32, tag="pz", bufs=1)
                    for k in range(kt):
                        nc.tensor.matmul(
                            pz,
                            mc(d_sb[:, ts(k, R)]),
                            mc(src_tiles[k][:, ts(n, NT)]),
                            start=(k == 0),
                            stop=(k == kt - 1),
                        )
                    nc.scalar.copy(z_sb[:, ts(n, NT)], pz)
                return z_sb

            # =================== layer 0 ===================
            z0 = lora_zT(d0_sb, KT0, xT, "z")
            h0T = []
            for m in range(MT0):
                w = pool.tile([P, KT0 * P], fmm, tag="W", bufs=4)
                nc.sync.dma_start(
                    out=w.rearrange("p (k c) -> p k c", c=P),
                    in_=k0[:, ts(m, P)].rearrange("(k p) c -> p k c", p=P),
                )
                ht = pool.tile([P, B], fmm, tag="B", bufs=16)
                h0T.append(ht)
                for n in range(NB):
                    ps = pp.tile([P, NT], f32, tag="pm", bufs=5)
                    for k in range(KT0):
                        nc.tensor.matmul(
                            ps,
                            mc(w[:, ts(k, P)]),
                            mc(xT[k][:, ts(n, NT)]),
                            start=(k == 0),
                            stop=False,
                        )
                    nc.tensor.matmul(
                        ps,
                        mc(u0_sb[:, ts(m, P)]),
                        mc(z0[:, ts(n, NT)]),
                        start=False,
                        stop=True,
                    )
                    nc.scalar.activation(
                        ht[:, ts(n, NT)], ps, AF.Relu, bias=b0_sb[:, ts(m, 1)]
                    )

            # =================== layer 1 ===================
            u1_sb = pool.tile([R, H2], fmm, tag="u", bufs=1)
            nc.sync.dma_start(out=u1_sb, in_=u1[:, :])
            z1 = lora_zT(d1_sb, KT1, h0T, "z")
            h1T = []
            for m in range(MT1):
                wa = pool.tile([P, 8 * P], fmm, tag="W", bufs=4)
                nc.sync.dma_start(
                    out=wa.rearrange("p (k c) -> p k c", c=P),
                    in_=k1[0:1024, ts(m, P)].rearrange("(k p) c -> p k c", p=P),
                )
                wb = pool.tile([P, 8 * P], fmm, tag="W", bufs=4)
                nc.sync.dma_start(
                    out=wb.rearrange("p (k c) -> p k c", c=P),
                    in_=k1[1024:2048, ts(m, P)].rearrange("(k p) c -> p k c", p=P),
                )
                ht = pool.tile([P, B], fmm, tag="A", bufs=16)
                h1T.append(ht)
                for n in range(NB):
                    ps = pp.tile([P, NT], f32, tag="pm", bufs=5)
                    for k in range(KT1):
                        wsrc = wa if k < 8 else wb
                        nc.tensor.matmul(
                            ps,
                            mc(wsrc[:, ts(k % 8, P)]),
                            mc(h0T[k][:, ts(n, NT)]),
                            start=(k == 0),
                            stop=False,
                        )
                    nc.tensor.matmul(
                        ps,
                        mc(u1_sb[:, ts(m, P)]),
                        mc(z1[:, ts(n, NT)]),
                        start=False,
                        stop=True,
                    )
                    nc.scalar.activation(
                        ht[:, ts(n, NT)], ps, AF.Relu, bias=b1_sb[:, ts(m, 1)]
                    )

            # =================== layer 2 (natural output) ===================
            u2_sb = pool.tile([R, H3], fmm, tag="u", bufs=1)
            nc.sync.dma_start(out=u2_sb, in_=u2[:, :])
            z2 = lora_zT(d2_sb, KT2, h1T, "z")
            out_acc = [None] * BT
            for g in range(KT2 // KG2):
                kg = []
                for j in range(KG2):
                    kt_ = pool.tile([P, H3], fmm, tag="E", bufs=8)
                    kg.append(kt_)
                    nc.sync.dma_start(out=kt_, in_=k2[ts(g * KG2 + j, P), :])
                for m in range(BT):
                    if g == 0:
                        out_acc[m] = pool.tile([P, H3], f32, tag="B", bufs=16, name=f"oacc{m}")
                    for n in range(N2):
                        ps = pp.tile([P, NT], f32, tag="pm", bufs=5)
                        first = True
                        if g == 0:
                            # bias broadcast over partitions: b2[m,n] += b2[n]
                            nc.tensor.matmul(
                                ps,
                                mc(ones),
                                mc(b2_sb[:, ts(n, NT)]),
                                start=True,
                                stop=False,
                            )
                            first = False
                        is_last = g == KT2 // KG2 - 1
                        for j in range(KG2):
                            k = g * KG2 + j
                            nc.tensor.matmul(
                                ps,
                                mc(h1T[k][:, ts(m, P)]),
                                mc(kg[j][:, ts(n, NT)]),
                                start=first,
                                stop=(not is_last) and j == KG2 - 1,
                            )
                            first = False
                        if is_last:
                            # rank-8 LoRA delta folded into the same PSUM group
                            nc.tensor.matmul(
                                ps,
                                mc(z2[:, ts(m, P)]),
                                mc(u2_sb[:, ts(n, NT)]),
                                start=False,
                                stop=True,
                            )
                        if g == 0:
                            nc.vector.tensor_copy(out_acc[m][:, ts(n, NT)], ps)
                        else:
                            nc.vector.tensor_add(
                                out_acc[m][:, ts(n, NT)],
                                out_acc[m][:, ts(n, NT)],
                                ps,
                            )
                for m in range(BT):
                    if g == KT2 // KG2 - 1:
                        nc.sync.dma_start(out=out[ts(m, P), :], in_=out_acc[m])

    if not nc.is_finalized():
        nc.finalize()
    return nc


def _get_nc():
    if "nc" not in _CACHE:
        _CACHE["nc"] = _build()
    return _CACHE["nc"]


def build_in_maps(inputs):
    def c(a):
        return np.ascontiguousarray(a, dtype=np.float32)

    in_maps = []
    for t in range(T):
        in_maps.append(
            {
                "x": c(inputs["x"][t]),
                "k0": c(inputs["k0"]),
                "b0": c(inputs["b0"]),
                "d0": c(inputs["d0"][:, :, t] * SCALING),
                "u0": c(inputs["u0"][:, :, t]),
                "k1": c(inputs["k1"]),
                "b1": c(inputs["b1"]),
                "d1": c(inputs["d1"][:, :, t] * SCALING),
                "u1": c(inputs["u1"][:, :, t]),
                "k2": c(inputs["k2"]),
                "b2": c(inputs["b2"]),
                "d2": c(inputs["d2"][:, :, t] * SCALING),
                "u2": c(inputs["u2"][:, :, t]),
            }
        )
    return in_maps


def kernel(**inputs):
    from concourse import bass_utils

    nc = _get_nc()
    in_maps = build_in_maps(inputs)
    res = bass_utils.run_bass_kernel_spmd(nc, in_maps, core_ids=list(range(T)))
    return np.stack([r["out"] for r in res.results], axis=0)



# revision 42
# speedup vs baseline: 1.1987x; 1.1987x over previous
"""Trainium2 Bass kernel for 3-layer per-task LoRA MLP.

Full-input contract: kernel(**inputs) takes the unsharded tensors and returns
the full [8, 1024, 1024] output. Internally the task axis (t=8) is sharded
across 8 NeuronCores (one task per core); base weights are replicated.

Per-core strategy:
  - base matmuls in bf16 (PSUM accumulates fp32): 1 cycle/row on the PE
  - the entire LoRA side-path (z = (s*d)^T h, the rank-8 up-projections, and
    layer-2's folded delta) runs in fp8e4m3 with DoubleRow perf mode:
    two K-halves contract per pass at 0.5 cycles/row. Scales keep fp8
    operands in range: u is shipped *16, z is stored /16
  - host pre-packs every tensor into per-partition-contiguous tile layouts
    (1 DMA descriptor per partition) and pre-transposes x, so the device does
    no transposes and few, large DMAs
  - activations live transposed in SBUF: h^T [feat(part), batch(free)];
    the final layer uses h2^T as the stationary operand producing natural
    [batch, feat] output directly
  - layer 2: delta2 = (s*d2)@u2 is folded into resident k2 on-device; its 32
    PSUM chunks are woven between layer-0 n=0 groups (a few run up front while
    x is in flight), staged by the Activation engine into the not-yet-used
    h1T tiles, and k2s += delta runs on the idle DVE after the n=0 pass.
    Layer-2 bias is added by the DVE during the PSUM->SBUF output copy
    against a host-broadcast bias tile.
"""

import sys

if "/opt/trn_rl_repo" not in sys.path:
    sys.path.insert(0, "/opt/trn_rl_repo")

import numpy as np

T, B, D = 8, 1024, 1024
H1, H2, H3 = 2048, 2048, 1024
R = 8
SCALING = 2.0   # alpha/rank = 16/8
ZS = 16.0       # fp8 scale balance: u *= ZS on host, z stored /ZS on device
P = 128
NT = 512        # PSUM free-dim tile (fp32 one-bank limit)

KT0 = D // P   # 8  k-tiles, layer 0
KT1 = H1 // P  # 16 k-tiles, layer 1
KT2 = H2 // P  # 16 k-tiles, layer 2
MT0 = H1 // P  # 16 m-tiles, layer 0
MT1 = H2 // P  # 16 m-tiles, layer 1
BT = B // P    # 8  batch 128-tiles
NB = B // NT   # 2  batch 512-halves
N2 = H3 // NT  # 2  feature 512-halves (layer 2)

_CACHE = {}


def _build():
    import concourse.bass as bass
    import concourse.mybir as mybir
    from concourse import bacc
    from concourse.tile import TileContext

    f32 = mybir.dt.float32
    bf16 = mybir.dt.bfloat16
    f8 = mybir.dt.float8e4
    AF = mybir.ActivationFunctionType
    DR = mybir.MatmulPerfMode.DoubleRow

    nc = bacc.Bacc(None, target_bir_lowering=False, name="lora_mlp")

    # host-packed dram tensors (see build_in_maps for layouts)
    xt = nc.dram_tensor("xt", (P, KT0 * B), bf16, kind="ExternalInput")
    xf = nc.dram_tensor("xf", (P, KT0 * B), f8, kind="ExternalInput")
    w0 = nc.dram_tensor("w0", (P, MT0 * KT0 * P), bf16, kind="ExternalInput")
    w1 = nc.dram_tensor("w1", (P, MT1 * KT1 * P), bf16, kind="ExternalInput")
    k2 = nc.dram_tensor("k2", (P, KT2 * H3), bf16, kind="ExternalInput")
    # uz: fp8 rank-pair packs [d2t-pairs | u2-pairs | u0-pairs | u1-pairs];
    # loaded into 4-row bands at partition bases 0/32/64 (matmul operands must
    # share a base in {0, 32, 64})
    uz = nc.dram_tensor("uz", (4, 2 * (H2 + H3 + H1 + H2)), f8, kind="ExternalInput")
    # dd: [d0 | d1] z-compute stationary packs (s*d, fp8). 64 cols per k-tile:
    # ranks 0-3 at cols 0-3, ranks 4-7 at cols 32-35, rest zero — dual-fp8
    # Ldweights needs >=16 columns, and the z halves then land at PSUM
    # partitions 0 and 32 (legal Activation read bases)
    dd = nc.dram_tensor("dd", (P, (KT0 + KT1) * 64), f8, kind="ExternalInput")
    # bb: [b0 | b1]; b2b: b2 broadcast across partitions (needed only at layer 2)
    bb = nc.dram_tensor("bb", (P, MT0 + MT1), f32, kind="ExternalInput")
    b2b = nc.dram_tensor("b2b", (P, H3), f32, kind="ExternalInput")
    out = nc.dram_tensor("out", (B, H3), f32, kind="ExternalOutput")

    with TileContext(nc) as tc:
        with (
            tc.tile_pool(name="main", bufs=1) as pool,
            tc.tile_pool(name="psum", bufs=1, space="PSUM") as pp,
        ):
            # resident tiles
            # xs cols: n*(KT0*NT) + k*NT + b   (n-half major so half 0 lands first)
            xs = pool.tile([P, KT0 * B], bf16, tag="xs", bufs=1)
            xf8 = pool.tile([P, KT0 * B], f8, tag="xf8", bufs=1)
            # w0s cols: m*(KT0*P) + k*P + c  (m-major: eighths match m-loop order)
            w0s = pool.tile([P, MT0 * KT0 * P], bf16, tag="w0", bufs=1)
            # k2s cols: k*H3 + f
            k2s = pool.tile([P, KT2 * H3], bf16, tag="k2", bufs=1)
            # band 0: [d2p | u2p]; band 32: [u0p | z0-store]; band 64: [u1p |
            # z1-store] — each matmul's operand pair shares a partition base
            UZW = 2 * H2 + 2 * B  # 6144
            uzs = pool.tile([68, UZW], f8, tag="uz", bufs=1)
            dds = pool.tile([P, (KT0 + KT1) * 64], f8, tag="dd", bufs=1)
            bbs = pool.tile([P, MT0 + MT1], f32, tag="bb", bufs=1)
            b2s = pool.tile([P, H3], f32, tag="b2", bufs=1)
            z08 = uzs[32:36, 2 * H1 : 2 * H1 + 2 * B]
            z18 = uzs[64:68, 2 * H2 : 2 * H2 + 2 * B]
            # fp8 copy of h0^T for the z1 matmuls; cols n*(KT1*NT) + k*NT + b
            h0f8 = pool.tile([P, KT1 * B], f8, tag="h0f8", bufs=1)

            b0s = bbs[:, 0:MT0]
            b1s = bbs[:, MT0 : MT0 + MT1]
            # rank-pair views [4, 2, H] from 4-row bands
            d2p = uzs[0:4, 0 : 2 * H2].rearrange("p (a c) -> p a c", a=2)
            u2p = uzs[0:4, 2 * H2 : 2 * H2 + 2 * H3].rearrange("p (a c) -> p a c", a=2)
            u0p = uzs[32:36, 0 : 2 * H1].rearrange("p (a c) -> p a c", a=2)
            u1p = uzs[64:68, 0 : 2 * H2].rearrange("p (a c) -> p a c", a=2)
            # k-pair views: [P, n_chunks, NT] with adjacent k-tiles along dim 1
            xf8v = xf8.rearrange("p (a b) -> p a b", b=NT)    # [128, 16, 512]
            h0f8v = h0f8.rearrange("p (a b) -> p a b", b=NT)  # [128, 32, 512]
            ddv = dds.rearrange("p (k c) -> p k c", c=64)     # [128, 24, 64]
            z08v = z08.rearrange("p (a b) -> p a b", a=2)     # [4, 2, 1024]
            z18v = z18.rearrange("p (a b) -> p a b", a=2)

            def xsl(k, n):  # bf16 xT moving slice [P, NT] for (k-tile, n-half)
                return xs[:, n * (KT0 * NT) + k * NT : n * (KT0 * NT) + (k + 1) * NT]

            # ---- upfront DMAs, ordered for earliest PE start ----
            S0 = 2 * H2 + 2 * H3  # 6144: band-0 section [d2p | u2p]
            nc.sync.dma_start(out=uzs[0:4, 0:S0], in_=uz[:, 0:S0])
            HB = KT0 * NT  # 4096: half of xs / xf8
            nc.sync.dma_start(out=xf8[:, 0:HB], in_=xf[:, 0:HB])
            nc.sync.dma_start(out=uzs[32:36, 0 : 2 * H1], in_=uz[:, S0 : S0 + 2 * H1])
            nc.sync.dma_start(
                out=uzs[64:68, 0 : 2 * H2], in_=uz[:, S0 + 2 * H1 : S0 + 2 * H1 + 2 * H2]
            )
            nc.sync.dma_start(out=dds, in_=dd[:, :])
            nc.sync.dma_start(out=xs[:, 0:HB], in_=xt[:, 0:HB])
            WQ = MT0 * KT0 * P // 8  # w0 eighth (2 m-tiles)
            nc.sync.dma_start(out=w0s[:, 0:WQ], in_=w0[:, 0:WQ])
            nc.sync.dma_start(out=bbs, in_=bb[:, :])
            for q in range(1, 8):
                nc.sync.dma_start(
                    out=w0s[:, q * WQ : (q + 1) * WQ], in_=w0[:, q * WQ : (q + 1) * WQ]
                )
            nc.sync.dma_start(out=xs[:, HB : 2 * HB], in_=xt[:, HB : 2 * HB])
            nc.sync.dma_start(out=xf8[:, HB : 2 * HB], in_=xf[:, HB : 2 * HB])
            KQ = KT2 * H3 // 4
            for q in range(4):
                nc.sync.dma_start(
                    out=k2s[:, q * KQ : (q + 1) * KQ], in_=k2[:, q * KQ : (q + 1) * KQ]
                )
            nc.sync.dma_start(out=b2s, in_=b2b[:, :])

            h0T = [
                pool.tile([P, B], bf16, tag="h0", bufs=16, name=f"h0T{i}")
                for i in range(MT0)
            ]
            h1T = [
                pool.tile([P, B], bf16, tag="h1", bufs=16, name=f"h1T{i}")
                for i in range(MT1)
            ]

            # delta2 chunk i (i = k*2+n): one DoubleRow matmul (both rank
            # halves in one pass), staged bf16 into h1T[k] with the 1/ZS
            # rescale (u2 ships *ZS). Staging alternates between the
            # Activation engine and the DVE so the PSUM ring drains at
            # matmul pace during the prefix.
            def delta2_chunk(i):
                k, n = i // N2, i % N2
                ps = pp.tile([P, NT], f32, tag="pm", bufs=5)
                nc.tensor.matmul(
                    ps,
                    d2p[:, :, k * P : (k + 1) * P],
                    u2p[:, :, n * NT : (n + 1) * NT],
                    start=True,
                    stop=True,
                    perf_mode=DR,
                )
                dst = h1T[k][:, n * NT : (n + 1) * NT]
                if i % 2 == 0:
                    nc.scalar.activation(dst, ps, AF.Copy, scale=1.0 / ZS)
                else:
                    nc.vector.tensor_scalar_mul(dst, ps, 1.0 / ZS)

            def zcalc(z8, z8v_unused, kbase, kt, pairs, n):
                """z^T[:, n-half] = (s*d)^T @ h via DoubleRow k-pair matmuls;
                the 64-col padded stationary puts rank halves at PSUM
                partitions 0 and 32; stored fp8 at 1/ZS scale."""
                pz = pp.tile([64, NT], f32, tag="pz", bufs=1)
                for kk in range(kt // 2):
                    nc.tensor.matmul(
                        pz,
                        ddv[:, kbase + 2 * kk : kbase + 2 * kk + 2, :],
                        pairs(kk, n),
                        start=(kk == 0),
                        stop=(kk == kt // 2 - 1),
                        perf_mode=DR,
                    )
                for i in range(2):
                    nc.scalar.activation(
                        z8[:, i * B + n * NT : i * B + (n + 1) * NT],
                        pz[32 * i : 32 * i + 4, :],
                        AF.Copy,
                        scale=1.0 / ZS,
                    )

            # several delta2 chunks run up front while x is still in flight
            NPRE = 24
            for i in range(NPRE):
                delta2_chunk(i)
            ndelta = NPRE

            # =================== layer 0 (n-outer; w0 fully resident) ===========
            for n in range(NB):
                zcalc(
                    z08,
                    None,
                    0,
                    KT0,
                    lambda kk, nn: xf8v[:, nn * KT0 + 2 * kk : nn * KT0 + 2 * kk + 2, :],
                    n,
                )
                for m in range(MT0):
                    ps = pp.tile([P, NT], f32, tag="pm", bufs=5)
                    wbase = m * KT0 * P
                    for k in range(KT0):
                        nc.tensor.matmul(
                            ps,
                            w0s[:, wbase + k * P : wbase + (k + 1) * P],
                            xsl(k, n),
                            start=(k == 0),
                            stop=False,
                        )
                    nc.tensor.matmul(
                        ps,
                        u0p[:, :, m * P : (m + 1) * P],
                        z08v[:, :, n * NT : (n + 1) * NT],
                        start=False,
                        stop=True,
                        perf_mode=DR,
                    )
                    nc.scalar.activation(
                        h0T[m][:, n * NT : (n + 1) * NT],
                        ps,
                        AF.Relu,
                        bias=b0s[:, m : m + 1],
                    )
                    # fp8 copy of h0 for z1 (DVE is idle during layer 0)
                    nc.vector.tensor_copy(
                        h0f8[:, n * (KT1 * NT) + m * NT : n * (KT1 * NT) + (m + 1) * NT],
                        h0T[m][:, n * NT : (n + 1) * NT],
                    )
                    for _ in range(1):  # weave remaining delta2 chunks, 1/group
                        if ndelta < KT2 * N2:
                            delta2_chunk(ndelta)
                            ndelta += 1
                zcalc(
                    z18,
                    None,
                    KT0,
                    KT1,
                    lambda kk, nn: h0f8v[
                        :, nn * KT1 + 2 * kk : nn * KT1 + 2 * kk + 2, :
                    ],
                    n,
                )
                if n == 0:
                    # k2s += staged delta2 (h1T); all 32 chunks are staged by
                    # n=0 m=10, and the k2 DMA lands well before these fire
                    for k in range(KT2):
                        sl2 = k2s[:, k * H3 : (k + 1) * H3]
                        nc.vector.tensor_add(sl2, sl2, h1T[k][:, :])

            # =================== layer 1 (m-outer; w1 streamed; z1/up1) ==========
            for m in range(MT1):
                wt = pool.tile([P, KT1 * P], bf16, tag="W1", bufs=4)
                nc.sync.dma_start(out=wt, in_=w1[:, m * KT1 * P : (m + 1) * KT1 * P])
                for n in range(NB):
                    ps = pp.tile([P, NT], f32, tag="pm", bufs=5)
                    for k in range(KT1):
                        nc.tensor.matmul(
                            ps,
                            wt[:, k * P : (k + 1) * P],
                            h0T[k][:, n * NT : (n + 1) * NT],
                            start=(k == 0),
                            stop=False,
                        )
                    nc.tensor.matmul(
                        ps,
                        u1p[:, :, m * P : (m + 1) * P],
                        z18v[:, :, n * NT : (n + 1) * NT],
                        start=False,
                        stop=True,
                        perf_mode=DR,
                    )
                    nc.scalar.activation(
                        h1T[m][:, n * NT : (n + 1) * NT],
                        ps,
                        AF.Relu,
                        bias=b1s[:, m : m + 1],
                    )

            # =================== layer 2 (natural output; k2_eff resident) ======
            HN = NT // 2
            for m in range(BT):
                for n in range(N2):
                    last = m == BT - 1 and n == N2 - 1
                    if not last:
                        ps = pp.tile([P, NT], f32, tag="pm", bufs=5)
                        for k in range(KT2):
                            nc.tensor.matmul(
                                ps,
                                h1T[k][:, m * P : (m + 1) * P],
                                k2s[:, k * H3 + n * NT : k * H3 + (n + 1) * NT],
                                start=(k == 0),
                                stop=(k == KT2 - 1),
                            )
                        ot = pool.tile([P, NT], f32, tag="out", bufs=3)
                        nc.vector.tensor_add(ot, ps, b2s[:, n * NT : (n + 1) * NT])
                        nc.sync.dma_start(
                            out=out[m * P : (m + 1) * P, n * NT : (n + 1) * NT],
                            in_=ot,
                        )
                    else:
                        # final group runs as two N=256 sub-groups so the first
                        # half's drain+DMA overlaps the second half's matmuls
                        for h in range(2):
                            c0 = n * NT + h * HN
                            psh = pp.tile([P, HN], f32, tag="pm2", bufs=2)
                            for k in range(KT2):
                                nc.tensor.matmul(
                                    psh,
                                    h1T[k][:, m * P : (m + 1) * P],
                                    k2s[:, k * H3 + c0 : k * H3 + c0 + HN],
                                    start=(k == 0),
                                    stop=(k == KT2 - 1),
                                )
                            oth = pool.tile([P, HN], f32, tag="out2", bufs=2)
                            nc.vector.tensor_add(oth, psh, b2s[:, c0 : c0 + HN])
                            nc.sync.dma_start(
                                out=out[m * P : (m + 1) * P, c0 : c0 + HN], in_=oth
                            )

    if not nc.is_finalized():
        nc.finalize()
    return nc


def _get_nc():
    if "nc" not in _CACHE:
        _CACHE["nc"] = _build()
    return _CACHE["nc"]


def build_in_maps(inputs):
    import ml_dtypes

    bf16 = ml_dtypes.bfloat16
    f8 = ml_dtypes.float8_e4m3fn

    def bf(a):
        return np.ascontiguousarray(a).astype(bf16)

    def f32(a):
        return np.ascontiguousarray(a, dtype=np.float32)

    def rpair(a):  # [8, H] -> [4, 2*H] rank-half pairs
        h = a.shape[1]
        return a.reshape(2, 4, h).transpose(1, 0, 2).reshape(4, 2 * h)

    # shared (task-independent) packs
    # w0[p, (m k c)] = k0[k*128+p, m*128+c]
    w0h = bf(
        np.asarray(inputs["k0"], np.float32)
        .reshape(KT0, P, MT0, P)
        .transpose(1, 2, 0, 3)
        .reshape(P, MT0 * KT0 * P)
    )
    # w1[p, (m k c)] = k1[k*128+p, m*128+c]
    w1h = bf(
        np.asarray(inputs["k1"], np.float32)
        .reshape(KT1, P, MT1, P)
        .transpose(1, 2, 0, 3)
        .reshape(P, MT1 * KT1 * P)
    )
    # k2[p, (k f)] = k2[k*128+p, f]
    k2h = bf(
        np.asarray(inputs["k2"], np.float32)
        .reshape(KT2, P, H3)
        .transpose(1, 0, 2)
        .reshape(P, KT2 * H3)
    )
    # bb = [b0 | b1] as [P, MT0 + MT1] f32
    bbh = f32(
        np.concatenate(
            [
                np.asarray(inputs["b0"], np.float32).reshape(MT0, P).T,
                np.asarray(inputs["b1"], np.float32).reshape(MT1, P).T,
            ],
            axis=1,
        )
    )
    b2h = f32(
        np.broadcast_to(np.asarray(inputs["b2"], np.float32)[None, :], (P, H3))
    )

    def dpack(d, kt):
        """[K, R] -> [p, (k c64)]: per k-tile 64 cols, ranks 0-3 at cols 0-3
        and ranks 4-7 at cols 32-35 (rest zero), pre-scaled."""
        dk = (
            (np.asarray(d, np.float32) * SCALING)
            .reshape(kt, P, R)
            .transpose(1, 0, 2)
        )  # [p, k, 8]
        out = np.zeros((P, kt, 64), np.float32)
        out[:, :, 0:4] = dk[:, :, 0:4]
        out[:, :, 32:36] = dk[:, :, 4:8]
        return out.reshape(P, kt * 64)

    in_maps = []
    for t in range(T):
        # xt[p, (n k b)] = x[t][n*512+b, k*128+p]
        xpack = (
            np.asarray(inputs["x"][t], np.float32)
            .T.reshape(KT0, P, NB, NT)
            .transpose(1, 2, 0, 3)
            .reshape(P, KT0 * B)
        )
        # uz = [d2t-pairs | u2-pairs*ZS | u0-pairs*ZS | u1-pairs*ZS] fp8
        uzh = np.concatenate(
            [
                rpair(np.asarray(inputs["d2"][:, :, t], np.float32).T * SCALING),
                rpair(np.asarray(inputs["u2"][:, :, t], np.float32) * ZS),
                rpair(np.asarray(inputs["u0"][:, :, t], np.float32) * ZS),
                rpair(np.asarray(inputs["u1"][:, :, t], np.float32) * ZS),
            ],
            axis=1,
        ).astype(f8)
        ddh = np.concatenate(
            [
                dpack(inputs["d0"][:, :, t], KT0),
                dpack(inputs["d1"][:, :, t], KT1),
            ],
            axis=1,
        ).astype(f8)
        in_maps.append(
            {
                "xt": bf(xpack),
                "xf": xpack.astype(f8),
                "w0": w0h,
                "w1": w1h,
                "k2": k2h,
                "uz": np.ascontiguousarray(uzh),
                "dd": np.ascontiguousarray(ddh),
                "bb": bbh,
                "b2b": b2h,
            }
        )
    return in_maps


def kernel(**inputs):
    from concourse import bass_utils

    nc = _get_nc()
    in_maps = build_in_maps(inputs)
    res = bass_utils.run_bass_kernel_spmd(nc, in_maps, core_ids=list(range(T)))
    return np.stack([r["out"] for r in res.results], axis=0)
